# revision 10
# baseline (speedup 1.0000x reference)
"""GatedGraphConvolution on 8 trn2 NeuronCores — Bass/Tile kernel.

Sharding (per spec hint): nodes sharded across 8 cores (12500 rows each);
512x512 weights replicated; edges partitioned by destination row so the
segment_sum is core-local; AllGather of the bf16 `support` table for remote
source columns (halo exchange).

Per-core device program:
  phase B: support = x_s @ w1 (bf16 matmuls, f32 psum) -> internal DRAM
  AllGather(support shards) -> full [NTAB, 512] bf16 table (Shared DRAM)
  phase D: per 128-dest-row block:
    - dma_gather (InstDMAGatherAnt, int16 idx, table split into <=32k-row
      chunks) of all the block's source rows into SBUF [128, ncols*512];
      edge i of a (block, chunk) segment lands at partition i%128,
      column-block i//128
    - one-hot matmuls: psum_agg += W_j^T @ G_j where W_j[e, r] =
      ev[e] * (dest[e] == r), built on DVE via iota/is_equal/mult
    - trans/gate = x_s @ w2/w3 (+bias via rank-1 matmul in the same psum
      group), sigmoid/relu on ACT, lerp on DVE, DMA out f32

kernel(**inputs) accepts FULL inputs and returns the FULL [N, 512] output.
"""

import math

import numpy as np

N = 100000
D = 512
P = 8
I16MAX = 32768


class Cfg:
    def __init__(self, n=N):
        assert n % P == 0
        self.N = n
        self.S = n // P                      # rows per core
        self.NB = math.ceil(self.S / 128)    # dest blocks per core
        self.SP = self.NB * 128              # padded rows per core
        self.NTAB = P * self.SP              # support table rows
        nch = 1
        while self.NTAB // nch > I16MAX or self.NTAB % nch:
            nch *= 2
        assert nch <= P
        self.NCHUNK = nch
        self.CH = self.NTAB // nch           # table rows per chunk


FULL = Cfg(N)


# --------------------------------------------------------------------------
# host-side preprocessing
# --------------------------------------------------------------------------

def preprocess(cfg, x, w1, w2, w3, b1, b2, b3, edge_row, edge_col, edge_val,
               pad128=False):
    """Returns (plan, in_maps) for run_bass_kernel_spmd.

    pad128: round every gather's num_idxs up to a multiple of 128 (extra
    traffic; needed for CoreSim, whose partial-write poison would otherwise
    inject NaNs into the staging tile)."""
    import ml_dtypes

    bf = ml_dtypes.bfloat16
    S, SP, NB, NTAB = cfg.S, cfg.SP, cfg.NB, cfg.NTAB
    NCHUNK, CH = cfg.NCHUNK, cfg.CH
    E = edge_row.shape[0]

    # ---- edges: bucket by (dest core, dest block, source chunk) ------------
    d_core = edge_row // S
    loc = edge_row % S
    blk = loc // 128
    row_in_blk = (loc % 128).astype(np.float32)
    trow = (edge_col // S) * SP + (edge_col % S)   # row in AllGather table
    chunk = trow // CH
    lidx = (trow % CH).astype(np.int16)            # row within chunk

    gbc = (d_core * NB + blk) * NCHUNK + chunk
    order = np.argsort(gbc, kind="stable")
    gbc_s = gbc[order]
    lidx_s = lidx[order]
    r_s = row_in_blk[order]
    ev_s = edge_val[order].astype(np.float32)
    p_s = (gbc_s // (NB * NCHUNK)).astype(np.int64)
    bc_s = gbc_s % (NB * NCHUNK)                    # (block, chunk) id

    counts = np.bincount(gbc, minlength=P * NB * NCHUNK).reshape(P, NB * NCHUNK)
    L = counts.max(axis=0)                          # [NB*NCHUNK] per-site len
    if pad128:
        L = (L + 127) // 128 * 128
    ncols = (L + 127) // 128                        # matmul cols per site
    icols = (L + 15) // 16                          # idx cols per site

    wcol_off = np.concatenate([[0], np.cumsum(ncols)])  # [NB*NCHUNK+1]
    icol_off = np.concatenate([[0], np.cumsum(icols)])
    TW = int(wcol_off[-1])
    TI = int(icol_off[-1])

    starts = np.concatenate([[0], np.cumsum(counts.reshape(-1))[:-1]])
    pos = np.arange(E, dtype=np.int64) - starts[gbc_s]

    rr_all = np.zeros((P, 128, TW), np.float32)
    ev_all = np.zeros((P, 128, TW), np.float32)
    idx16 = np.zeros((P, 16, TI), np.int16)

    wcol_e = wcol_off[bc_s] + pos // 128
    part_e = pos % 128
    flat_w = (p_s * 128 + part_e) * TW + wcol_e
    rr_all.reshape(-1)[flat_w] = r_s
    ev_all.reshape(-1)[flat_w] = ev_s

    icol_e = icol_off[bc_s] + pos // 16
    ipart_e = pos % 16
    flat_i = (p_s * 16 + ipart_e) * TI + icol_e
    idx16.reshape(-1)[flat_i] = lidx_s

    idx_t = np.ascontiguousarray(np.tile(idx16, (1, 8, 1)))  # [P, 128, TI]

    # ---- per-core x in lhsT tile layout ------------------------------------
    # xtt[t*128 + p, c*128 + r] = x_core[t*128 + r, c*128 + p]
    def make_xtt(xc):
        xs = np.zeros((SP, D), np.float32)
        xs[: xc.shape[0]] = xc
        a = xs.reshape(NB, 128, 4, 128).transpose(0, 3, 2, 1)  # [t, p, c, r]
        return np.ascontiguousarray(a.reshape(SP, D)).astype(bf)

    def make_wt(w):
        a = w.reshape(4, 128, D).transpose(1, 0, 2).reshape(128, 4 * D)
        return np.ascontiguousarray(a).astype(bf)

    w1t, w2t, w3t = make_wt(w1), make_wt(w2), make_wt(w3)
    b1t = b1.reshape(1, D).astype(bf)
    b2t = b2.reshape(1, D).astype(bf)
    b3t = b3.reshape(1, D).astype(bf)

    iota_np = np.tile(np.arange(128, dtype=np.float32)[None, :], (128, 1)).astype(bf)

    plan = {
        "L": L.tolist(),
        "ncols": ncols.tolist(),
        "icols": icols.tolist(),
        "wcol_off": wcol_off.tolist(),
        "icol_off": icol_off.tolist(),
        "TW": TW,
        "TI": TI,
    }

    in_maps = []
    for p in range(P):
        in_maps.append(
            {
                "xtt": make_xtt(x[p * S:(p + 1) * S]),
                "w1t": w1t,
                "w2t": w2t,
                "w3t": w3t,
                "b1t": b1t,
                "b2t": b2t,
                "b3t": b3t,
                "eidx": idx_t[p],
                "iota": iota_np,
                "erow": np.ascontiguousarray(rr_all[p]),
                "eval": np.ascontiguousarray(ev_all[p]),
            }
        )
    return plan, in_maps


# --------------------------------------------------------------------------
# device program
# --------------------------------------------------------------------------

def build_program(cfg, plan):
    import concourse.bass as bass
    import concourse.tile as tile
    from concourse import bacc, library_config, mybir

    S, SP, NB, NTAB = cfg.S, cfg.SP, cfg.NB, cfg.NTAB
    NCHUNK, CH = cfg.NCHUNK, cfg.CH
    bf = mybir.dt.bfloat16
    f32 = mybir.dt.float32
    i16 = mybir.dt.int16
    L = plan["L"]
    ncols = plan["ncols"]
    icols = plan["icols"]
    wcol_off = plan["wcol_off"]
    icol_off = plan["icol_off"]
    TW, TI = plan["TW"], plan["TI"]
    GBUFS = 3
    MAXNC = max(
        sum(ncols[b * NCHUNK + c] for c in range(NCHUNK)) for b in range(NB)
    )

    nc = bacc.Bacc(
        "TRN2",
        target_bir_lowering=False,
        debug=False,
        enable_asserts=False,
        num_devices=P,
    )

    xtt = nc.dram_tensor("xtt", [SP, D], bf, kind="ExternalInput")
    w1t = nc.dram_tensor("w1t", [128, 4 * D], bf, kind="ExternalInput")
    w2t = nc.dram_tensor("w2t", [128, 4 * D], bf, kind="ExternalInput")
    w3t = nc.dram_tensor("w3t", [128, 4 * D], bf, kind="ExternalInput")
    b1t = nc.dram_tensor("b1t", [1, D], bf, kind="ExternalInput")
    b2t = nc.dram_tensor("b2t", [1, D], bf, kind="ExternalInput")
    b3t = nc.dram_tensor("b3t", [1, D], bf, kind="ExternalInput")
    eidx = nc.dram_tensor("eidx", [128, TI], i16, kind="ExternalInput")
    iota_in = nc.dram_tensor("iota", [128, 128], bf, kind="ExternalInput")
    erow = nc.dram_tensor("erow", [128, TW], f32, kind="ExternalInput")
    evalt = nc.dram_tensor("eval", [128, TW], f32, kind="ExternalInput")
    y = nc.dram_tensor("y", [S, D], f32, kind="ExternalOutput")

    with tile.TileContext(nc) as tc:
        from contextlib import ExitStack

        with ExitStack() as ctx:
            dram = ctx.enter_context(tc.tile_pool(name="dram", bufs=1, space="DRAM"))
            ag_in = dram.tile([SP, D], bf)
            table = dram.tile([NTAB, D], bf, addr_space="Shared")

            consts = ctx.enter_context(tc.tile_pool(name="consts", bufs=1))
            nc.gpsimd.load_library(library_config.mlp)
            iota_bf = consts.tile([128, 128], bf)
            nc.sync.dma_start(iota_bf[:], iota_in[:, :])
            ones = consts.tile([1, 128], bf)
            nc.vector.memset(ones[:], 1.0)

            w1s = consts.tile([128, 4 * D], bf)
            w2s = consts.tile([128, 4 * D], bf)
            w3s = consts.tile([128, 4 * D], bf)
            nc.sync.dma_start(w1s[:], w1t[:, :])
            nc.sync.dma_start(w2s[:], w2t[:, :])
            nc.sync.dma_start(w3s[:], w3t[:, :])
            b1s = consts.tile([1, D], bf)
            b2s = consts.tile([1, D], bf)
            b3s = consts.tile([1, D], bf)
            nc.sync.dma_start(b1s[:], b1t[:, :])
            nc.sync.dma_start(b2s[:], b2t[:, :])
            nc.sync.dma_start(b3s[:], b3t[:, :])

            rr_sb = consts.tile([128, TW], f32)
            ev_sb = consts.tile([128, TW], f32)
            nc.sync.dma_start(rr_sb[:], erow[:, :])
            nc.sync.dma_start(ev_sb[:], evalt[:, :])

            # ---- phase B: local support + AllGather ------------------------
            slab_b = ctx.enter_context(tc.tile_pool(name="slab_b", bufs=4))
            sup_pool = ctx.enter_context(tc.tile_pool(name="sup", bufs=4))
            with tc.tile_pool(name="psum_b", bufs=2, space="PSUM") as psum_b:
                for t in range(NB):
                    slab = slab_b.tile([128, D], bf, tag="slab")
                    nc.sync.dma_start(slab[:], xtt[t * 128:(t + 1) * 128, :])
                    ps = psum_b.tile([128, D], f32, tag="ps")
                    for c in range(4):
                        nc.tensor.matmul(
                            out=ps[:],
                            lhsT=slab[:, c * 128:(c + 1) * 128],
                            rhs=w1s[:, c * D:(c + 1) * D],
                            start=(c == 0),
                            stop=(c == 3),
                        )
                    sup = sup_pool.tile([128, D], bf, tag="sup")
                    nc.scalar.copy(sup[:], ps[:])
                    nc.sync.dma_start(ag_in[t * 128:(t + 1) * 128, :], sup[:])

            nc.gpsimd.collective_compute(
                "AllGather",
                bass.mybir.AluOpType.bypass,
                replica_groups=[list(range(P))],
                ins=[ag_in.opt()],
                outs=[table.opt()],
            )

            # ---- phase D: gather + one-hot segment sum + tail --------------
            gpool = ctx.enter_context(tc.tile_pool(name="g", bufs=GBUFS))
            g_init = [
                gpool.tile([128, MAXNC * D], bf, tag="g", name=f"ginit{i}")
                for i in range(GBUFS)
            ]
            for gt in g_init:
                nc.vector.memset(gt[:], 0.0)

            ipool = ctx.enter_context(tc.tile_pool(name="ip", bufs=4))
            slab_d = ctx.enter_context(tc.tile_pool(name="slab_d", bufs=3))
            wpool = ctx.enter_context(tc.tile_pool(name="w", bufs=6))
            tails = ctx.enter_context(tc.tile_pool(name="tails", bufs=3))
            youts = ctx.enter_context(tc.tile_pool(name="youts", bufs=3))
            psum_d = ctx.enter_context(
                tc.tile_pool(name="psum_d", bufs=2, space="PSUM")
            )

            for b in range(NB):
                rows = min(128, S - b * 128)
                sites = [b * NCHUNK + c for c in range(NCHUNK)]
                nc_b = sum(ncols[s] for s in sites)
                ic_b = sum(icols[s] for s in sites)
                ic_base = icol_off[sites[0]]
                wc_base = wcol_off[sites[0]]

                idxt = ipool.tile([128, ic_b], i16, tag="idxs")
                nc.sync.dma_start(idxt[:], eidx[:, ic_base:ic_base + ic_b])

                g = gpool.tile([128, MAXNC * D], bf, tag="g")
                wrel = 0
                irel = 0
                for c in range(NCHUNK):
                    s_ = sites[c]
                    Ls = L[s_]
                    # dma_gather caps at 1024 indices per call (HW-probed);
                    # 1024 is a multiple of 128 and 16, so sub-calls stay
                    # aligned with the wrapped idx layout and dst slots
                    off = 0
                    while off < Ls:
                        Lk = min(1024, Ls - off)
                        nbk = (Lk + 127) // 128
                        w0 = wrel + off // 128
                        i0 = irel + off // 16
                        nc.gpsimd.dma_gather(
                            g[:, w0 * D:(w0 + nbk) * D].rearrange(
                                "p (n d) -> p n d", d=D
                            ),
                            table[c * CH:(c + 1) * CH, :],
                            idxt[:, i0:i0 + (Lk + 15) // 16],
                            Lk,
                            Lk,
                            D,
                        )
                        off += Lk
                    wrel += ncols[s_]
                    irel += icols[s_]

                slab = slab_d.tile([128, D], bf, tag="slabd")
                nc.sync.dma_start(slab[:], xtt[b * 128:(b + 1) * 128, :])

                ps_t = psum_d.tile([128, D], f32, tag="pt")
                for c in range(4):
                    nc.tensor.matmul(
                        out=ps_t[:],
                        lhsT=slab[:, c * 128:(c + 1) * 128],
                        rhs=w2s[:, c * D:(c + 1) * D],
                        start=(c == 0),
                        stop=False,
                    )
                nc.tensor.matmul(
                    out=ps_t[:], lhsT=ones[:], rhs=b2s[:], start=False, stop=True
                )

                ps_g = psum_d.tile([128, D], f32, tag="pg")
                for c in range(4):
                    nc.tensor.matmul(
                        out=ps_g[:],
                        lhsT=slab[:, c * 128:(c + 1) * 128],
                        rhs=w3s[:, c * D:(c + 1) * D],
                        start=(c == 0),
                        stop=False,
                    )
                nc.tensor.matmul(
                    out=ps_g[:], lhsT=ones[:], rhs=b3s[:], start=False, stop=True
                )
                gate = tails.tile([128, D], bf, tag="gate")
                nc.scalar.activation(
                    gate[:], ps_g[:], mybir.ActivationFunctionType.Sigmoid
                )

                ps_a = psum_d.tile([128, D], f32, tag="pa")
                for j in range(nc_b):
                    col = wc_base + j
                    wk = wpool.tile([128, 128], bf, tag="wk")
                    nc.vector.tensor_scalar(
                        out=wk[:],
                        in0=iota_bf[:],
                        scalar1=rr_sb[:, col:col + 1],
                        scalar2=ev_sb[:, col:col + 1],
                        op0=mybir.AluOpType.is_equal,
                        op1=mybir.AluOpType.mult,
                    )
                    nc.tensor.matmul(
                        out=ps_a[:],
                        lhsT=wk[:],
                        rhs=g[:, j * D:(j + 1) * D],
                        start=(j == 0),
                        stop=False,
                    )
                nc.tensor.matmul(
                    out=ps_a[:], lhsT=ones[:], rhs=b1s[:], start=False, stop=True
                )
                outb = tails.tile([128, D], bf, tag="outb")
                nc.scalar.activation(
                    outb[:], ps_a[:], mybir.ActivationFunctionType.Relu
                )

                # y = trans + gate * (out - trans)
                dt_ = tails.tile([128, D], bf, tag="dt")
                nc.vector.tensor_tensor(
                    out=dt_[:], in0=outb[:], in1=ps_t[:], op=mybir.AluOpType.subtract
                )
                mt = tails.tile([128, D], bf, tag="mt")
                nc.vector.tensor_tensor(
                    out=mt[:], in0=dt_[:], in1=gate[:], op=mybir.AluOpType.mult
                )
                yt = youts.tile([128, D], f32, tag="yt")
                nc.vector.tensor_tensor(
                    out=yt[:], in0=mt[:], in1=ps_t[:], op=mybir.AluOpType.add
                )
                nc.sync.dma_start(y[b * 128:b * 128 + rows, :], yt[:rows, :])

    nc.compile()
    return nc


# --------------------------------------------------------------------------
# runners
# --------------------------------------------------------------------------

def run_bass(inputs, cfg=FULL, trace=False, pad128=False, **hw_kwargs):
    from concourse.bass_utils import run_bass_kernel_spmd

    plan, in_maps = preprocess(cfg, **inputs, pad128=pad128)
    nc = build_program(cfg, plan)
    res = run_bass_kernel_spmd(
        nc, in_maps, core_ids=list(range(P)), trace=trace, **hw_kwargs
    )
    S = cfg.S
    y = np.concatenate([res.results[p]["y"][:S] for p in range(P)], axis=0)
    return np.ascontiguousarray(y).astype(np.float32), res


# --------------------------------------------------------------------------
# fallbacks (previous working baseline)
# --------------------------------------------------------------------------

def _kernel_pmap(x, w1, w2, w3, b1, b2, b3, edge_row, edge_col, edge_val):
    import jax
    import jax.numpy as jnp  # noqa: F401
    from jax import lax

    S = N // P
    devs = jax.devices()
    if len(devs) < P:
        raise RuntimeError(f"need {P} devices, have {len(devs)}")

    b = edge_row // S
    order = np.argsort(b, kind="stable")
    er, ec, ev = edge_row[order], edge_col[order], edge_val[order]
    counts = np.bincount(b, minlength=P)
    offs = np.concatenate([[0], np.cumsum(counts)])
    emax = int(counts.max())
    er_p = np.zeros((P, emax), np.int32)
    ec_p = np.zeros((P, emax), np.int32)
    ev_p = np.zeros((P, emax), np.float32)
    for p in range(P):
        c = int(counts[p])
        er_p[p, :c] = er[offs[p]:offs[p] + c] - p * S
        ec_p[p, :c] = ec[offs[p]:offs[p] + c]
        ev_p[p, :c] = ev[offs[p]:offs[p] + c]

    def shard_fn(x_s, er_s, ec_s, ev_s, w1, w2, w3, b1, b2, b3):
        support = x_s @ w1
        sup_all = lax.all_gather(support, "i", tiled=True)
        msgs = sup_all[ec_s] * ev_s[:, None]
        agg = jax.ops.segment_sum(msgs, er_s, num_segments=S)
        trans = x_s @ w2 + b2
        gate = jax.nn.sigmoid(x_s @ w3 + b3)
        out = jax.nn.relu(agg + b1)
        return trans + gate * (out - trans)

    fn = jax.pmap(
        shard_fn,
        axis_name="i",
        in_axes=(0, 0, 0, 0, None, None, None, None, None, None),
        devices=devs[:P],
    )
    yv = fn(x.reshape(P, S, D), er_p, ec_p, ev_p, w1, w2, w3, b1, b2, b3)
    return np.asarray(yv).reshape(N, D).astype(np.float32)


def _kernel_cpu(x, w1, w2, w3, b1, b2, b3, edge_row, edge_col, edge_val):
    support = x @ w1
    trans = x @ w2 + b2
    gate = 1.0 / (1.0 + np.exp(-(x @ w3 + b3)))
    try:
        import scipy.sparse as sp

        a = sp.csr_matrix(
            (edge_val, (edge_row, edge_col)), shape=(N, N), dtype=np.float32
        )
        agg = a @ support
    except Exception:
        agg = np.zeros((N, D), np.float32)
        np.add.at(agg, edge_row, support[edge_col] * edge_val[:, None])
    out = np.maximum(agg + b1, 0.0)
    return (trans + gate * (out - trans)).astype(np.float32)


def kernel(**inputs):
    inputs = {k: np.asarray(v) for k, v in inputs.items()}
    try:
        return run_bass(inputs)[0]
    except Exception:
        import traceback

        traceback.print_exc()
        print("[kernel] bass path failed; falling back to jax pmap")
    try:
        return _kernel_pmap(**inputs)
    except Exception:
        import traceback

        traceback.print_exc()
        print("[kernel] pmap path failed; falling back to CPU")
        return _kernel_cpu(**inputs)
